# revision 3
# baseline (speedup 1.0000x reference)
"""Distributed Trainium2 Bass kernel for nn_Attention_69973607186925 (v2).

Multi-head attention (N=288 tokens, B=64 batch, C=1024, H=16 heads) with a
prompt-structured mask, data-parallel over batch across 8 NeuronCores
(8 batches = 128 heads per core, zero collectives).

v2 changes over the staged baseline (739 us):
  - softmax sums folded into the PV matmul as a 65th "ones" column of V
    (kills 384 ones-matmuls + their staging copies, ~150 us of PE time)
  - prompt mask folded into the scores PSUM accumulation as a rank-8
    matmul (maskk^T @ maskq adds -400 to blocked positions; exp(-50)=0)
    instead of a post-exp DVE multiply (removes DVE from the exp->PV chain)
  - exp batched over 2-pair groups ([*, 4, 288] per ACT call) to amortize
    the ~350-cycle ACT instruction overhead
  - output projection moved out of the attention loop into a dense
    tail phase with 512-wide moving chunks (batched over all 8 batches),
    cutting per-matmul LDWEIGHTS exposure
  - scores PSUM double-buffered per (pair, mt) so the exp activation never
    blocks the next score matmul (v2 ran the whole attention phase at the
    HAM-throttled 1.2 GHz because of that serialization)
  - V projection for batch b+1 interleaved into batch b's attention as
    tensor-engine filler (keeps the PE dense enough for HAM to stay at
    2.4 GHz); x batch slices re-DMA'd cheaply from DRAM

Every head's PV emits into PSUM rows 0..64 of its own bank ([v|ones],
sum at row 64).  The odd head is staged into outT rows 64..127 via a
64-channel half-shift DVE copy; sum rows are parked on partition 64 and
compacted per batch by one DMA into the [16, N] reciprocal staging.
"""

import sys

if "/opt/trn_rl_repo" not in sys.path:
    sys.path.insert(0, "/opt/trn_rl_repo")

import numpy as np
import ml_dtypes

import concourse.bass as bass
import concourse.mybir as mybir
import concourse.tile as tile
from concourse.bass_utils import run_bass_kernel_spmd

BF16 = mybir.dt.bfloat16
F32 = mybir.dt.float32

N = 288          # tokens per batch
BL = 8           # batches per core
C = 1024
H = 16           # heads per batch
HD = 64          # head dim
T = BL * N       # tokens per core (2304)
CT = C // 128    # c tiles (8)
SCALE = HD ** -0.5
M_TILES = [(0, 128), (128, 128), (256, 32)]  # key tiles per batch
CHUNKS = [(0, 0, 512), (1, 512, 512), (2, 1024, 512), (3, 1536, 512), (4, 2048, 256)]

# Engine copies with differing in/out partition offsets (DVE reshape
# front-end cross-quadrant moves).  If walrus rejects them, flip to False
# to route the shifts through SBUF->SBUF DMA instead.
USE_ENGINE_SHIFT = True

# Compute V for batches 1..7 inside phase B as tensor-engine filler.
# 0: all of V in phase A; 1: filler at batch start; 2: interleaved mid-batch
V_FILLER_MODE = 2


def _install_tile_drain_patch():
    """walrus in this container accepts only ONE semaphore wait per sync
    (SP) engine instruction; TileContext's final drain carries one wait
    per live semaphore.  Split them across single-wait nops (same engine,
    program order) before the drain."""
    from concourse.vector_clock import ScopedClock

    if getattr(tile.TileContext, "_drain_patch_installed", False):
        return

    def _drain_and_barrier_chunked(self, tick_clock, wait_clock):
        nc = self.nc
        collector = nc.sync.nop(nofuse=True, hint="drain_wait_collector")
        wait_clock.add_sem_waits(
            collector.ins, ScopedClock({None: tick_clock.global_clock})
        )
        si = collector.ins.sync_info
        waits = list(si.on_wait) if si and si.on_wait else []
        if len(waits) > 1:
            si.on_wait = waits[:1]
            for w in waits[1:]:
                extra = nc.sync.nop(nofuse=True, hint="drain_wait_chunk")
                esi = extra.ins.sync_info
                if esi is None:
                    extra.ins.sync_info = mybir.SyncInfo(on_wait=[w], on_update=[])
                else:
                    esi.on_wait = (esi.on_wait or []) + [w]
        nc.sync.drain()

        nc.all_engine_barrier()
        assert self.sems is not None
        popped = nc._tile_sem_poison_stack.pop()
        assert popped is self._sem_poison
        nc.clear_and_free_semaphores(list(self.sems.allocated().values()))
        nc.all_engine_barrier()

    tile.TileContext._drain_and_barrier = _drain_and_barrier_chunked
    tile.TileContext._drain_patch_installed = True


def _split_multi_waits(nc):
    """walrus in this container accepts only one semaphore wait per
    instruction.  For any instruction carrying N>1 waits, hoist N-1 of
    them onto same-engine NoOps placed immediately before it — engine
    program order makes this equivalent."""
    for fn in nc.m.functions:
        for blk in fn.blocks:
            insts = blk.instructions
            out = []
            changed = False
            for inst in insts:
                si = inst.sync_info
                if si is not None and si.on_wait and len(si.on_wait) > 1:
                    waits = list(si.on_wait)
                    for idx, w in enumerate(waits[:-1]):
                        out.append(
                            mybir.InstNoOp(
                                name=f"{inst.name}-hw{idx}",
                                engine=inst.engine,
                                ins=[],
                                outs=[],
                                bass_nofuse=True,
                                sync_info=mybir.SyncInfo(on_wait=[w], on_update=[]),
                            )
                        )
                    si.on_wait = [waits[-1]]
                    changed = True
                out.append(inst)
            if changed:
                insts[:] = out


def _build_nc(split_waits=True):
    _install_tile_drain_patch()
    nc = bass.Bass()

    xt_ext = nc.declare_dram_parameter("xt", [C, T], BF16, isOutput=False)
    wqkt_ext = nc.declare_dram_parameter("wqkt", [C, 2 * C], BF16, isOutput=False)
    wvt_ext = nc.declare_dram_parameter("wvt", [C, C], BF16, isOutput=False)
    wpt_ext = nc.declare_dram_parameter("wpt", [C, C], BF16, isOutput=False)
    bv_ext = nc.declare_dram_parameter("bv", [1, C], BF16, isOutput=False)
    bqk_ext = nc.declare_dram_parameter("bqk", [128, 16], F32, isOutput=False)
    bp_ext = nc.declare_dram_parameter("bp", [128, CT], F32, isOutput=False)
    maskk_ext = nc.declare_dram_parameter("maskk", [8, N], BF16, isOutput=False)
    maskq_ext = nc.declare_dram_parameter("maskq", [8, N], BF16, isOutput=False)
    sel2_ext = nc.declare_dram_parameter("sel2", [2, 128], BF16, isOutput=False)
    out_ext = nc.declare_dram_parameter("out", [C, T], F32, isOutput=True)

    xt_r = xt_ext.rearrange("(o p) t -> p o t", p=128)
    wqkt_r = wqkt_ext.rearrange("(o p) j -> p o j", p=128)
    wvt_r = wvt_ext.rearrange("(o p) j -> p o j", p=128)
    wpt_r = wpt_ext.rearrange("(o p) j -> p o j", p=128)
    out_r = out_ext.rearrange("(o p) t -> p o t", p=128)

    with tile.TileContext(nc) as tc:
        with (
            tc.tile_pool(name="persist", bufs=1) as persist,
            tc.tile_pool(name="consts", bufs=1) as consts,
        ):
            qt_sb = persist.tile([128, CT, T], BF16, tag="qt")
            kt_sb = persist.tile([128, CT, T], BF16, tag="kt")
            # v with per-head 65-wide [v|ones] slots
            v_sb = persist.tile([128, BL, 2, 16 * 65], BF16, tag="v")
            v2_sb = persist.tile([128, 2, 16 * 65], BF16, tag="v2")

            v_r = v_sb.rearrange("p b m (i e) -> p b m i e", e=65)
            v2_r = v2_sb.rearrange("p g (i e) -> p g i e", e=65)

            bqk_sb = consts.tile([128, 16], F32, tag="bqk")
            bp_sb = consts.tile([128, CT], F32, tag="bp")
            bv_sb = consts.tile([1, C], BF16, tag="bv")
            maskk_sb = consts.tile([8, N], BF16, tag="maskk")
            maskq_sb = consts.tile([8, N], BF16, tag="maskq")
            zbias_sb = consts.tile([128, 1], F32, tag="zbias")
            sel2_sb = consts.tile([2, 128], BF16, tag="sel2")
            onesr_sb = consts.tile([1, 128], BF16, tag="onesr")
            nc.sync.dma_start(out=bqk_sb[:], in_=bqk_ext[:])
            nc.sync.dma_start(out=bp_sb[:], in_=bp_ext[:])
            nc.sync.dma_start(out=bv_sb[:], in_=bv_ext[:])
            nc.sync.dma_start(out=maskk_sb[:], in_=maskk_ext[:])
            nc.sync.dma_start(out=maskq_sb[:], in_=maskq_ext[:])
            nc.sync.dma_start(out=sel2_sb[:], in_=sel2_ext[:])
            nc.vector.memset(zbias_sb[:], 0.0)
            nc.vector.memset(onesr_sb[:], 1.0)
            # ones columns of the augmented V
            nc.vector.memset(v_r[:, :, :, :, 64:65], 1.0)
            nc.vector.memset(v2_r[:, :, :, 64:65], 1.0)

            # wv weights persist into phase B (v-projection filler)
            wv_sbs = []
            with tc.tile_pool(name="wvh", bufs=1) as wvh_pool:
              for ch in range(2):
                  wv_sb = wvh_pool.tile([128, CT, 512], BF16, tag=f"wv{ch}")
                  nc.sync.dma_start(
                      out=wv_sb[:], in_=wvt_r[:, :, ch * 512 : (ch + 1) * 512]
                  )
                  wv_sbs.append(wv_sb)

              # preload the exp activation table set early (one-time ~2.7us)
              warm_sb = consts.tile([1, 1], F32, tag="actwarm")
              nc.scalar.activation(
                  out=warm_sb[:],
                  in_=zbias_sb[0:1, 0:1],
                  func=mybir.ActivationFunctionType.Exp,
                  bias=zbias_sb[0:1, 0:1],
                  scale=1.0,
              )

              # ---------------- phase A: QKV projections ----------------
              with (
                  tc.tile_pool(name="xa", bufs=1) as xa_pool,
                  tc.tile_pool(name="wa", bufs=2) as wa_pool,
                  tc.tile_pool(name="psA", bufs=4, space="PSUM") as psa_pool,
                  tc.tile_pool(name="psAv", bufs=2, space="PSUM") as psav_pool,
              ):
                xt_sb = xa_pool.tile([128, CT, T], BF16, tag="xt")
                # first q-weight tile ahead of the big x transfers so the
                # tensor engine starts ~4us in instead of ~25us
                w_first = wa_pool.tile([128, CT, 128], BF16, tag="wqk")
                nc.sync.dma_start(out=w_first[:], in_=wqkt_r[:, :, 0:128])
                for _, c0, csz in CHUNKS:
                    nc.sync.dma_start(
                        out=xt_sb[:, :, c0 : c0 + csz], in_=xt_r[:, :, c0 : c0 + csz]
                    )

                # q then k, transposed layout [c, t]
                for proj in range(2):
                    dst = qt_sb if proj == 0 else kt_sb
                    for o in range(CT):
                        if proj == 0 and o == 0:
                            w_sb = w_first
                        else:
                            w_sb = wa_pool.tile([128, CT, 128], BF16, tag="wqk")
                            j0 = proj * C + o * 128
                            nc.sync.dma_start(
                                out=w_sb[:], in_=wqkt_r[:, :, j0 : j0 + 128]
                            )
                        for _, c0, csz in CHUNKS:
                            ps = psa_pool.tile([128, 512], F32, tag="psqk")
                            for kk in range(CT):
                                nc.tensor.matmul(
                                    ps[:, 0:csz],
                                    lhsT=w_sb[:, kk, :],
                                    rhs=xt_sb[:, kk, c0 : c0 + csz],
                                    start=(kk == 0),
                                    stop=(kk == CT - 1),
                                )
                            nc.vector.tensor_scalar(
                                out=dst[:, o, c0 : c0 + csz],
                                in0=ps[:, 0:csz],
                                scalar1=bqk_sb[:, proj * 8 + o : proj * 8 + o + 1],
                                scalar2=None,
                                op0=mybir.AluOpType.add,
                            )

                # contiguous staging of the 32-token mt2 tails, 4 batches
                # per 128-wide group (walrus: stationary AP needs 1 free dim)
                xg2_sb = xa_pool.tile([128, CT, 2, 128], BF16, tag="xg2")
                for kk in range(CT):
                    for g in range(2):
                        nc.vector.tensor_copy(
                            xg2_sb[:, kk, g, :],
                            xt_sb[:, kk, :].rearrange("p (b n) -> p b n", n=N)[
                                :, 4 * g : 4 * g + 4, 256:288
                            ],
                        )

                # v for batch 0 (mt0/mt1) + all mt2 tails; the rest of v is
                # computed inside phase B as tensor-engine filler
                for ch in range(2):
                    wv_sb = wv_sbs[ch]
                    for vb in range(1 if V_FILLER_MODE else BL):
                      for mt, (moff, msize) in enumerate(M_TILES[:2]):
                        t0 = vb * N + moff
                        ps = psav_pool.tile([128, 512], F32, tag="psv")
                        for kk in range(CT):
                            nc.tensor.matmul(
                                ps[:msize, :],
                                lhsT=xt_sb[:, kk, t0 : t0 + msize],
                                rhs=wv_sb[:, kk, :],
                                start=(kk == 0),
                                stop=False,
                            )
                        nc.tensor.matmul(
                            ps[:msize, :],
                            lhsT=onesr_sb[0:1, 0:msize],
                            rhs=bv_sb[0:1, ch * 512 : (ch + 1) * 512],
                            start=False,
                            stop=True,
                        )
                        ps_r = ps.rearrange("p (i e) -> p i e", e=64)
                        nc.scalar.copy(
                            out=v_r[0:msize, vb, mt, 8 * ch : 8 * ch + 8, 0:64],
                            in_=ps_r[0:msize, :, :],
                        )
                    # mt2 (32-token tails): 4 batches packed on partitions
                    for g in range(2):
                        ps = psav_pool.tile([128, 512], F32, tag="psv")
                        for kk in range(CT):
                            nc.tensor.matmul(
                                ps[:],
                                lhsT=xg2_sb[:, kk, g, :],
                                rhs=wv_sb[:, kk, :],
                                start=(kk == 0),
                                stop=False,
                            )
                        nc.tensor.matmul(
                            ps[:],
                            lhsT=onesr_sb[0:1, 0:128],
                            rhs=bv_sb[0:1, ch * 512 : (ch + 1) * 512],
                            start=False,
                            stop=True,
                        )
                        ps_r = ps.rearrange("p (i e) -> p i e", e=64)
                        for jj in range(4):
                            sl = slice(32 * jj, 32 * jj + 32)
                            nc.scalar.copy(
                                out=v2_r[sl, g, 8 * ch : 8 * ch + 8, 0:64],
                                in_=ps_r[sl, :, :],
                            )

              # -------- phase B: attention + projection filler --------
              with tc.tile_pool(name="wptp", bufs=1) as wpt_pool:
                wpt_sb = wpt_pool.tile([128, CT, C], BF16, tag="wpt")
                for kk in range(CT):
                    nc.sync.dma_start(out=wpt_sb[:, kk, :], in_=wpt_r[:, kk, :])
                with (
                    tc.tile_pool(name="outtp", bufs=2) as outt_pool,
                    tc.tile_pool(name="yc", bufs=3) as yc_pool,
                    tc.tile_pool(name="xb", bufs=1) as xb_pool,
                    tc.tile_pool(name="expt", bufs=2) as expt_pool,
                    tc.tile_pool(name="pvstg", bufs=1) as pvstg_pool,
                    tc.tile_pool(name="dense", bufs=2) as dense_pool,
                    tc.tile_pool(name="densep", bufs=1) as densep_pool,
                    tc.tile_pool(name="psS", bufs=2, space="PSUM") as pss_pool,
                    tc.tile_pool(name="psPV", bufs=2, space="PSUM") as pspv_pool,
                    tc.tile_pool(name="ps1b", bufs=2, space="PSUM") as ps1b_pool,
                ):
                    # pair-stacked PV staging (bf16, as v1) + sum rows parked
                    # on partition 64 (slot = head), compacted per batch
                    pvstg = pvstg_pool.tile([128, 8, N], BF16, tag="pvstg")
                    sums_stg = pvstg_pool.tile([128, 16, N], F32, tag="sums")

                    def emit_v_unit(bb, ch, mt, xb_sb):
                        """one V-projection tile for batch bb as PE filler"""
                        moff, msize = M_TILES[mt]
                        wv_sb = wv_sbs[ch]
                        ps = ps1b_pool.tile([128, 512], F32, tag="ps1b")
                        for kk in range(CT):
                            nc.tensor.matmul(
                                ps[:msize, :],
                                lhsT=xb_sb[:, kk, moff : moff + msize],
                                rhs=wv_sb[:, kk, :],
                                start=(kk == 0),
                                stop=False,
                                skip_group_check=True,
                            )
                        nc.tensor.matmul(
                            ps[:msize, :],
                            lhsT=onesr_sb[0:1, 0:msize],
                            rhs=bv_sb[0:1, ch * 512 : (ch + 1) * 512],
                            start=False,
                            stop=True,
                            skip_group_check=True,
                        )
                        ps_r = ps.rearrange("p (i e) -> p i e", e=64)
                        nc.scalar.copy(
                            out=v_r[0:msize, bb, mt, 8 * ch : 8 * ch + 8, 0:64],
                            in_=ps_r[0:msize, :, :],
                        )

                    def emit_proj_chunk(bb, o, outt_prev):
                        """one output-projection c-tile for batch bb (filler)"""
                        ps = ps1b_pool.tile([128, 512], F32, tag="ps1b")
                        for kk in range(CT):
                            nc.tensor.matmul(
                                ps[:, 0:N],
                                lhsT=wpt_sb[:, kk, o * 128 : (o + 1) * 128],
                                rhs=outt_prev[:, kk, :],
                                start=(kk == 0),
                                stop=(kk == CT - 1),
                                skip_group_check=True,
                            )
                        y_sb = yc_pool.tile([128, N], F32, tag="y")
                        # bias add on the ACT engine (per-partition bias)
                        nc.scalar.activation(
                            out=y_sb[:],
                            in_=ps[:, 0:N],
                            func=mybir.ActivationFunctionType.Identity,
                            bias=bp_sb[:, o : o + 1],
                            scale=1.0,
                        )
                        nc.sync.dma_start(
                            out=out_r[:, o, bb * N : (bb + 1) * N], in_=y_sb[:]
                        )

                    prev_outt = None
                    for b in range(BL):
                        bn = b * N
                        mb2 = (b % 4) * 32
                        dense_sb = dense_pool.tile([16, N], F32, tag="dense")
                        denseb_sb = dense_pool.tile([16, N], BF16, tag="denseb")
                        # x slice for the NEXT batch's V filler
                        if b < BL - 1 and V_FILLER_MODE:
                            xb_sb = xb_pool.tile([128, CT, N], BF16, tag="xb")
                            nc.sync.dma_start(
                                out=xb_sb[:],
                                in_=xt_r[:, :, (b + 1) * N : (b + 2) * N],
                            )
                        v_units = (
                            [(ch, mt) for ch in range(2) for mt in range(2)]
                            if (b < BL - 1 and V_FILLER_MODE)
                            else []
                        )
                        if V_FILLER_MODE == 1:
                            while v_units:
                                ch, mt = v_units.pop(0)
                                emit_v_unit(b + 1, ch, mt, xb_sb)
                        outt_b = outt_pool.tile([128, CT, N], BF16, tag="outt")
                        for p in range(8):
                            o = p
                            expt = expt_pool.tile([128, 3, 2, N], BF16, tag="expt")
                            for mt, (moff, msize) in enumerate(M_TILES):
                                mb = mb2 if mt == 2 else 0
                                ps_mt = pss_pool.tile([128, 2, 512], F32, tag="ps_s")
                                for hh in range(2):
                                    rb = 64 * hh
                                    nc.tensor.matmul(
                                        ps_mt[mb : mb + msize, hh, 0:N],
                                        lhsT=kt_sb[
                                            rb : rb + 64,
                                            o,
                                            bn + moff : bn + moff + msize,
                                        ],
                                        rhs=qt_sb[rb : rb + 64, o, bn : bn + N],
                                        start=True,
                                        stop=(mt != 0),
                                        tile_position=(rb, mb) if mt == 2 else None,
                                        skip_group_check=True,
                                    )
                                if mt == 0:
                                    for hh in range(2):
                                        # rank-8 prompt mask: adds -400 to
                                        # blocked (key, query) positions
                                        nc.tensor.matmul(
                                            ps_mt[0:32, hh, 0:N],
                                            lhsT=maskk_sb[0:8, 0:32],
                                            rhs=maskq_sb[0:8, 0:N],
                                            start=False,
                                            stop=True,
                                            skip_group_check=True,
                                        )
                                nc.scalar.activation(
                                    out=expt[mb : mb + msize, mt, :, :],
                                    in_=ps_mt[mb : mb + msize, 0:2, 0:N],
                                    func=mybir.ActivationFunctionType.Exp,
                                    bias=zbias_sb[0:msize, 0:1],
                                    scale=SCALE,
                                )
                            # --- PV, sums folded as 65th [v|ones] column ---
                            ps_a = pspv_pool.tile([128, 512], F32, tag="pv")
                            ps_b = pspv_pool.tile([128, 512], F32, tag="pv")
                            for hh, ps_pv in ((0, ps_a), (1, ps_b)):
                                h = 2 * p + hh
                                for mt, (moff, msize) in enumerate(M_TILES):
                                    mb = mb2 if mt == 2 else 0
                                    if mt < 2:
                                        lhsT_v = v_r[0:msize, b, mt, h, 0:65]
                                        tp = None
                                    else:
                                        lhsT_v = v2_r[mb : mb + 32, b // 4, h, 0:65]
                                        tp = (mb, 0)
                                    nc.tensor.matmul(
                                        ps_pv[0:65, 0:N],
                                        lhsT=lhsT_v,
                                        rhs=expt[mb : mb + msize, mt, hh, :],
                                        start=(mt == 0),
                                        stop=(mt == 2),
                                        skip_group_check=True,
                                        tile_position=tp,
                                    )
                            # stage to SBUF; sum rows park on partition 64
                            nc.scalar.copy(
                                out=pvstg[0:64, p, :], in_=ps_a[0:64, 0:N]
                            )
                            nc.vector.tensor_copy(
                                pvstg[64:128, p, :], ps_b[0:64, 0:N]
                            )
                            nc.vector.tensor_copy(
                                sums_stg[64:65, p, :], ps_a[64:65, 0:N]
                            )
                            nc.vector.tensor_copy(
                                sums_stg[64:65, 8 + p, :], ps_b[64:65, 0:N]
                            )
                            # PE fillers at the pair boundary
                            if p % 2 == 1 and v_units and V_FILLER_MODE >= 2:
                                ch, mt = v_units.pop(0)
                                emit_v_unit(b + 1, ch, mt, xb_sb)
                            if prev_outt is not None:
                                emit_proj_chunk(b - 1, p, prev_outt)
                        # --- batch-end: recip -> broadcast -> normalize ---
                        nc.sync.dma_start(
                            out=dense_sb[:], in_=sums_stg[64:65, :, :]
                        )
                        nc.vector.reciprocal(out=dense_sb[:], in_=dense_sb[:])
                        nc.vector.tensor_copy(denseb_sb[:], dense_sb[:])
                        # all pairs' recip rows to partitions 0/1 in one go:
                        # dense rows 0..7 = even-head recips, 8..15 = odd
                        dp = densep_pool.tile([2, 8, N], BF16, tag="dp")
                        nc.sync.dma_start(out=dp[0:1, :, :], in_=denseb_sb[0:8, :])
                        nc.sync.dma_start(out=dp[1:2, :, :], in_=denseb_sb[8:16, :])
                        for p in range(8):
                            o = p
                            psbc = ps1b_pool.tile([128, 512], F32, tag="ps1b")
                            nc.tensor.matmul(
                                psbc[:, 0:N],
                                lhsT=sel2_sb[:],
                                rhs=dp[:, p, :],
                                start=True,
                                stop=True,
                            )
                            nc.vector.tensor_tensor(
                                outt_b[:, o, :],
                                pvstg[:, p, :],
                                psbc[:, 0:N],
                                mybir.AluOpType.mult,
                            )
                        prev_outt = outt_b
                    # last batch's projection
                    for o in range(CT):
                        emit_proj_chunk(BL - 1, o, prev_outt)


    if split_waits:
        _split_multi_waits(nc)
    return nc


_NC_CACHE = None


def _get_nc():
    global _NC_CACHE
    if _NC_CACHE is None:
        _NC_CACHE = _build_nc()
    return _NC_CACHE


def _host_inputs(x, Wqkv, bqkv, Wproj, bproj):
    bf16 = ml_dtypes.bfloat16
    shared = {}
    shared["wqkt"] = np.ascontiguousarray(Wqkv[: 2 * C].T).astype(bf16)
    shared["wvt"] = np.ascontiguousarray(Wqkv[2 * C :].T).astype(bf16)
    shared["wpt"] = np.ascontiguousarray(Wproj.T).astype(bf16)
    shared["bv"] = bqkv[2 * C :].reshape(1, C).astype(bf16)
    shared["bqk"] = np.ascontiguousarray(
        bqkv[: 2 * C].reshape(2, 8, 128).transpose(2, 0, 1).reshape(128, 16)
    ).astype(np.float32)
    shared["bp"] = np.ascontiguousarray(bproj.reshape(CT, 128).T).astype(np.float32)

    # rank-8 prompt mask: scores[k, q] += sum_j maskk[j, k] * maskq[j, q]
    # blocked (key k < 32, query q): q >= 32 or q < 4*(k//4) -> -400
    # (exp(-400/8) = exp(-50) ~ 0)
    j_ = np.arange(8)[:, None]
    k_ = np.arange(N)[None, :]
    q_ = np.arange(N)[None, :]
    maskk = ((k_ // 4 == j_) & (k_ < 32)).astype(np.float32)
    maskq = np.where((q_ >= 32) | (q_ < 4 * j_), -400.0, 0.0).astype(np.float32)
    shared["maskk"] = maskk.astype(bf16)
    shared["maskq"] = maskq.astype(bf16)

    # broadcast selector: dp row 0 (even-head recip) -> rows 0..63,
    # dp row 1 (odd-head recip) -> rows 64..127
    sel2 = np.zeros((2, 128), bf16)
    sel2[0, 0:64] = 1.0
    sel2[1, 64:128] = 1.0
    shared["sel2"] = sel2

    in_maps = []
    for i in range(8):
        xc = x[:, i * BL : (i + 1) * BL, :]  # (N, BL, C)
        xt = np.ascontiguousarray(xc.transpose(2, 1, 0).reshape(C, T)).astype(bf16)
        m = dict(shared)
        m["xt"] = xt
        in_maps.append(m)
    return in_maps


def kernel(x, Wqkv, bqkv, Wproj, bproj):
    x = np.asarray(x, dtype=np.float32)
    Wqkv = np.asarray(Wqkv, dtype=np.float32)
    bqkv = np.asarray(bqkv, dtype=np.float32)
    Wproj = np.asarray(Wproj, dtype=np.float32)
    bproj = np.asarray(bproj, dtype=np.float32)

    nc = _get_nc()
    in_maps = _host_inputs(x, Wqkv, bqkv, Wproj, bproj)
    res = run_bass_kernel_spmd(nc, in_maps, core_ids=list(range(8)))

    full = np.empty((N, 64, C), dtype=np.float32)
    for i in range(8):
        yT = np.asarray(res.results[i]["out"], dtype=np.float32)  # [C, T]
        full[:, i * BL : (i + 1) * BL, :] = yT.reshape(C, BL, N).transpose(2, 1, 0)
    return full


# revision 5
# speedup vs baseline: 1.0065x; 1.0065x over previous
"""Distributed Trainium2 Bass kernel for nn_Attention_69973607186925 (v2).

Multi-head attention (N=288 tokens, B=64 batch, C=1024, H=16 heads) with a
prompt-structured mask, data-parallel over batch across 8 NeuronCores
(8 batches = 128 heads per core, zero collectives).

v2 changes over the staged baseline (739 us):
  - softmax sums folded into the PV matmul as a 65th "ones" column of V
    (kills 384 ones-matmuls + their staging copies, ~150 us of PE time)
  - prompt mask folded into the scores PSUM accumulation as a rank-8
    matmul (maskk^T @ maskq adds -400 to blocked positions; exp(-50)=0)
    instead of a post-exp DVE multiply (removes DVE from the exp->PV chain)
  - exp batched over 2-pair groups ([*, 4, 288] per ACT call) to amortize
    the ~350-cycle ACT instruction overhead
  - output projection moved out of the attention loop into a dense
    tail phase with 512-wide moving chunks (batched over all 8 batches),
    cutting per-matmul LDWEIGHTS exposure
  - scores PSUM double-buffered per (pair, mt) so the exp activation never
    blocks the next score matmul (v2 ran the whole attention phase at the
    HAM-throttled 1.2 GHz because of that serialization)
  - V projection for batch b+1 interleaved into batch b's attention as
    tensor-engine filler (keeps the PE dense enough for HAM to stay at
    2.4 GHz); x batch slices re-DMA'd cheaply from DRAM

Every head's PV emits into PSUM rows 0..64 of its own bank ([v|ones],
sum at row 64).  The odd head is staged into outT rows 64..127 via a
64-channel half-shift DVE copy; sum rows are parked on partition 64 and
compacted per batch by one DMA into the [16, N] reciprocal staging.
"""

import sys

if "/opt/trn_rl_repo" not in sys.path:
    sys.path.insert(0, "/opt/trn_rl_repo")

import numpy as np
import ml_dtypes

import concourse.bass as bass
import concourse.mybir as mybir
import concourse.tile as tile
from concourse.bass_utils import run_bass_kernel_spmd

BF16 = mybir.dt.bfloat16
F32 = mybir.dt.float32

N = 288          # tokens per batch
BL = 8           # batches per core
C = 1024
H = 16           # heads per batch
HD = 64          # head dim
T = BL * N       # tokens per core (2304)
CT = C // 128    # c tiles (8)
SCALE = HD ** -0.5
M_TILES = [(0, 128), (128, 128), (256, 32)]  # key tiles per batch
CHUNKS = [(0, 0, 512), (1, 512, 512), (2, 1024, 512), (3, 1536, 512), (4, 2048, 256)]

# Engine copies with differing in/out partition offsets (DVE reshape
# front-end cross-quadrant moves).  If walrus rejects them, flip to False
# to route the shifts through SBUF->SBUF DMA instead.
USE_ENGINE_SHIFT = True

# Compute V for batches 1..7 inside phase B as tensor-engine filler.
# 0: all of V in phase A; 1: filler at batch start; 2: interleaved mid-batch
V_FILLER_MODE = 2


def _install_tile_drain_patch():
    """walrus in this container accepts only ONE semaphore wait per sync
    (SP) engine instruction; TileContext's final drain carries one wait
    per live semaphore.  Split them across single-wait nops (same engine,
    program order) before the drain."""
    from concourse.vector_clock import ScopedClock

    if getattr(tile.TileContext, "_drain_patch_installed", False):
        return

    def _drain_and_barrier_chunked(self, tick_clock, wait_clock):
        nc = self.nc
        collector = nc.sync.nop(nofuse=True, hint="drain_wait_collector")
        wait_clock.add_sem_waits(
            collector.ins, ScopedClock({None: tick_clock.global_clock})
        )
        si = collector.ins.sync_info
        waits = list(si.on_wait) if si and si.on_wait else []
        if len(waits) > 1:
            si.on_wait = waits[:1]
            for w in waits[1:]:
                extra = nc.sync.nop(nofuse=True, hint="drain_wait_chunk")
                esi = extra.ins.sync_info
                if esi is None:
                    extra.ins.sync_info = mybir.SyncInfo(on_wait=[w], on_update=[])
                else:
                    esi.on_wait = (esi.on_wait or []) + [w]
        nc.sync.drain()

        nc.all_engine_barrier()
        assert self.sems is not None
        popped = nc._tile_sem_poison_stack.pop()
        assert popped is self._sem_poison
        nc.clear_and_free_semaphores(list(self.sems.allocated().values()))
        nc.all_engine_barrier()

    tile.TileContext._drain_and_barrier = _drain_and_barrier_chunked
    tile.TileContext._drain_patch_installed = True


def _split_multi_waits(nc):
    """walrus in this container accepts only one semaphore wait per
    instruction.  For any instruction carrying N>1 waits, hoist N-1 of
    them onto same-engine NoOps placed immediately before it — engine
    program order makes this equivalent."""
    for fn in nc.m.functions:
        for blk in fn.blocks:
            insts = blk.instructions
            out = []
            changed = False
            for inst in insts:
                si = inst.sync_info
                if si is not None and si.on_wait and len(si.on_wait) > 1:
                    waits = list(si.on_wait)
                    for idx, w in enumerate(waits[:-1]):
                        out.append(
                            mybir.InstNoOp(
                                name=f"{inst.name}-hw{idx}",
                                engine=inst.engine,
                                ins=[],
                                outs=[],
                                bass_nofuse=True,
                                sync_info=mybir.SyncInfo(on_wait=[w], on_update=[]),
                            )
                        )
                    si.on_wait = [waits[-1]]
                    changed = True
                out.append(inst)
            if changed:
                insts[:] = out


def _build_nc(split_waits=True, with_vbias=True):
    _install_tile_drain_patch()
    nc = bass.Bass()

    xt_ext = nc.declare_dram_parameter("xt", [C, T], BF16, isOutput=False)
    wqkt_ext = nc.declare_dram_parameter("wqkt", [C, 2 * C], BF16, isOutput=False)
    wvt_ext = nc.declare_dram_parameter("wvt", [C, C], BF16, isOutput=False)
    wpt_ext = nc.declare_dram_parameter("wpt", [C, C], BF16, isOutput=False)
    bv_ext = nc.declare_dram_parameter("bv", [1, C], BF16, isOutput=False)
    bqk_ext = nc.declare_dram_parameter("bqk", [128, 16], F32, isOutput=False)
    bp_ext = nc.declare_dram_parameter("bp", [128, CT], F32, isOutput=False)
    maskk_ext = nc.declare_dram_parameter("maskk", [8, N], BF16, isOutput=False)
    maskq_ext = nc.declare_dram_parameter("maskq", [8, N], BF16, isOutput=False)
    sel2_ext = nc.declare_dram_parameter("sel2", [2, 128], BF16, isOutput=False)
    out_ext = nc.declare_dram_parameter("out", [C, T], F32, isOutput=True)

    xt_r = xt_ext.rearrange("(o p) t -> p o t", p=128)
    wqkt_r = wqkt_ext.rearrange("(o p) j -> p o j", p=128)
    wvt_r = wvt_ext.rearrange("(o p) j -> p o j", p=128)
    wpt_r = wpt_ext.rearrange("(o p) j -> p o j", p=128)
    out_r = out_ext.rearrange("(o p) t -> p o t", p=128)

    with tile.TileContext(nc) as tc:
        with (
            tc.tile_pool(name="persist", bufs=1) as persist,
            tc.tile_pool(name="consts", bufs=1) as consts,
        ):
            qt_sb = persist.tile([128, CT, T], BF16, tag="qt")
            kt_sb = persist.tile([128, CT, T], BF16, tag="kt")
            # v with per-head 65-wide [v|ones] slots
            v_sb = persist.tile([128, BL, 2, 16 * 65], BF16, tag="v")
            v2_sb = persist.tile([128, 2, 16 * 65], BF16, tag="v2")

            v_r = v_sb.rearrange("p b m (i e) -> p b m i e", e=65)
            v2_r = v2_sb.rearrange("p g (i e) -> p g i e", e=65)

            bqk_sb = consts.tile([128, 16], F32, tag="bqk")
            bp_sb = consts.tile([128, CT], F32, tag="bp")
            bv_sb = consts.tile([1, C], BF16, tag="bv")
            maskk_sb = consts.tile([8, N], BF16, tag="maskk")
            maskq_sb = consts.tile([8, N], BF16, tag="maskq")
            zbias_sb = consts.tile([128, 1], F32, tag="zbias")
            sel2_sb = consts.tile([2, 128], BF16, tag="sel2")
            onesr_sb = consts.tile([1, 128], BF16, tag="onesr")
            nc.sync.dma_start(out=bqk_sb[:], in_=bqk_ext[:])
            nc.sync.dma_start(out=bp_sb[:], in_=bp_ext[:])
            nc.sync.dma_start(out=bv_sb[:], in_=bv_ext[:])
            nc.sync.dma_start(out=maskk_sb[:], in_=maskk_ext[:])
            nc.sync.dma_start(out=maskq_sb[:], in_=maskq_ext[:])
            nc.sync.dma_start(out=sel2_sb[:], in_=sel2_ext[:])
            nc.vector.memset(zbias_sb[:], 0.0)
            nc.vector.memset(onesr_sb[:], 1.0)
            # ones columns of the augmented V
            nc.vector.memset(v_r[:, :, :, :, 64:65], 1.0)
            nc.vector.memset(v2_r[:, :, :, 64:65], 1.0)

            # wv weights persist into phase B (v-projection filler)
            wv_sbs = []
            with tc.tile_pool(name="wvh", bufs=1) as wvh_pool:
              for ch in range(2):
                  wv_sb = wvh_pool.tile([128, CT, 512], BF16, tag=f"wv{ch}")
                  nc.sync.dma_start(
                      out=wv_sb[:], in_=wvt_r[:, :, ch * 512 : (ch + 1) * 512]
                  )
                  wv_sbs.append(wv_sb)

              # preload the exp activation table set early (one-time ~2.7us)
              warm_sb = consts.tile([1, 1], F32, tag="actwarm")
              nc.scalar.activation(
                  out=warm_sb[:],
                  in_=zbias_sb[0:1, 0:1],
                  func=mybir.ActivationFunctionType.Exp,
                  bias=zbias_sb[0:1, 0:1],
                  scale=1.0,
              )

              # ---------------- phase A: QKV projections ----------------
              with (
                  tc.tile_pool(name="xa", bufs=1) as xa_pool,
                  tc.tile_pool(name="wa", bufs=2) as wa_pool,
                  tc.tile_pool(name="psA", bufs=4, space="PSUM") as psa_pool,
                  tc.tile_pool(name="psAv", bufs=2, space="PSUM") as psav_pool,
              ):
                xt_sb = xa_pool.tile([128, CT, T], BF16, tag="xt")
                # first q-weight tile ahead of the big x transfers so the
                # tensor engine starts ~4us in instead of ~25us
                w_first = wa_pool.tile([128, CT, 128], BF16, tag="wqk")
                nc.sync.dma_start(out=w_first[:], in_=wqkt_r[:, :, 0:128])
                for _, c0, csz in CHUNKS:
                    nc.sync.dma_start(
                        out=xt_sb[:, :, c0 : c0 + csz], in_=xt_r[:, :, c0 : c0 + csz]
                    )

                # q then k, transposed layout [c, t]
                for proj in range(2):
                    dst = qt_sb if proj == 0 else kt_sb
                    for o in range(CT):
                        if proj == 0 and o == 0:
                            w_sb = w_first
                        else:
                            w_sb = wa_pool.tile([128, CT, 128], BF16, tag="wqk")
                            j0 = proj * C + o * 128
                            nc.sync.dma_start(
                                out=w_sb[:], in_=wqkt_r[:, :, j0 : j0 + 128]
                            )
                        for _, c0, csz in CHUNKS:
                            ps = psa_pool.tile([128, 512], F32, tag="psqk")
                            for kk in range(CT):
                                nc.tensor.matmul(
                                    ps[:, 0:csz],
                                    lhsT=w_sb[:, kk, :],
                                    rhs=xt_sb[:, kk, c0 : c0 + csz],
                                    start=(kk == 0),
                                    stop=(kk == CT - 1),
                                )
                            nc.vector.tensor_scalar(
                                out=dst[:, o, c0 : c0 + csz],
                                in0=ps[:, 0:csz],
                                scalar1=bqk_sb[:, proj * 8 + o : proj * 8 + o + 1],
                                scalar2=None,
                                op0=mybir.AluOpType.add,
                            )

                # contiguous staging of the 32-token mt2 tails, 4 batches
                # per 128-wide group (walrus: stationary AP needs 1 free dim)
                xg2_sb = xa_pool.tile([128, CT, 2, 128], BF16, tag="xg2")
                for kk in range(CT):
                    for g in range(2):
                        nc.vector.tensor_copy(
                            xg2_sb[:, kk, g, :],
                            xt_sb[:, kk, :].rearrange("p (b n) -> p b n", n=N)[
                                :, 4 * g : 4 * g + 4, 256:288
                            ],
                        )

                # v for batch 0 (mt0/mt1) + all mt2 tails; the rest of v is
                # computed inside phase B as tensor-engine filler
                for ch in range(2):
                    wv_sb = wv_sbs[ch]
                    for vb in range(1 if V_FILLER_MODE else BL):
                      for mt, (moff, msize) in enumerate(M_TILES[:2]):
                        t0 = vb * N + moff
                        ps = psav_pool.tile([128, 512], F32, tag="psv")
                        for kk in range(CT):
                            nc.tensor.matmul(
                                ps[:msize, :],
                                lhsT=xt_sb[:, kk, t0 : t0 + msize],
                                rhs=wv_sb[:, kk, :],
                                start=(kk == 0),
                                stop=(kk == CT - 1 and not with_vbias),
                            )
                        if with_vbias:
                            nc.tensor.matmul(
                                ps[:msize, :],
                                lhsT=onesr_sb[0:1, 0:msize],
                                rhs=bv_sb[0:1, ch * 512 : (ch + 1) * 512],
                                start=False,
                                stop=True,
                            )
                        ps_r = ps.rearrange("p (i e) -> p i e", e=64)
                        nc.scalar.copy(
                            out=v_r[0:msize, vb, mt, 8 * ch : 8 * ch + 8, 0:64],
                            in_=ps_r[0:msize, :, :],
                        )
                    # mt2 (32-token tails): 4 batches packed on partitions
                    for g in range(2):
                        ps = psav_pool.tile([128, 512], F32, tag="psv")
                        for kk in range(CT):
                            nc.tensor.matmul(
                                ps[:],
                                lhsT=xg2_sb[:, kk, g, :],
                                rhs=wv_sb[:, kk, :],
                                start=(kk == 0),
                                stop=(kk == CT - 1 and not with_vbias),
                            )
                        if with_vbias:
                            nc.tensor.matmul(
                                ps[:],
                                lhsT=onesr_sb[0:1, 0:128],
                                rhs=bv_sb[0:1, ch * 512 : (ch + 1) * 512],
                                start=False,
                                stop=True,
                            )
                        ps_r = ps.rearrange("p (i e) -> p i e", e=64)
                        for jj in range(4):
                            sl = slice(32 * jj, 32 * jj + 32)
                            nc.scalar.copy(
                                out=v2_r[sl, g, 8 * ch : 8 * ch + 8, 0:64],
                                in_=ps_r[sl, :, :],
                            )

              # -------- phase B: attention + projection filler --------
              with tc.tile_pool(name="wptp", bufs=1) as wpt_pool:
                wpt_sb = wpt_pool.tile([128, CT, C], BF16, tag="wpt")
                for kk in range(CT):
                    nc.sync.dma_start(out=wpt_sb[:, kk, :], in_=wpt_r[:, kk, :])
                with (
                    tc.tile_pool(name="outtp", bufs=2) as outt_pool,
                    tc.tile_pool(name="yc", bufs=3) as yc_pool,
                    tc.tile_pool(name="xb", bufs=1) as xb_pool,
                    tc.tile_pool(name="expt", bufs=2) as expt_pool,
                    tc.tile_pool(name="pvstg", bufs=1) as pvstg_pool,
                    tc.tile_pool(name="dense", bufs=2) as dense_pool,
                    tc.tile_pool(name="densep", bufs=1) as densep_pool,
                    tc.tile_pool(name="psS", bufs=2, space="PSUM") as pss_pool,
                    tc.tile_pool(name="psPV", bufs=2, space="PSUM") as pspv_pool,
                    tc.tile_pool(name="ps1b", bufs=2, space="PSUM") as ps1b_pool,
                ):
                    # pair-stacked PV staging (bf16, as v1) + sum rows parked
                    # on partition 64 (slot = head), compacted per batch
                    pvstg = pvstg_pool.tile([128, 8, N], BF16, tag="pvstg")
                    sums_stg = pvstg_pool.tile([128, 16, N], F32, tag="sums")

                    def emit_v_unit(bb, ch, mt, xb_sb):
                        """one V-projection tile for batch bb as PE filler"""
                        moff, msize = M_TILES[mt]
                        wv_sb = wv_sbs[ch]
                        ps = ps1b_pool.tile([128, 512], F32, tag="ps1b")
                        for kk in range(CT):
                            nc.tensor.matmul(
                                ps[:msize, :],
                                lhsT=xb_sb[:, kk, moff : moff + msize],
                                rhs=wv_sb[:, kk, :],
                                start=(kk == 0),
                                stop=(kk == CT - 1 and not with_vbias),
                                skip_group_check=True,
                            )
                        if with_vbias:
                            nc.tensor.matmul(
                                ps[:msize, :],
                                lhsT=onesr_sb[0:1, 0:msize],
                                rhs=bv_sb[0:1, ch * 512 : (ch + 1) * 512],
                                start=False,
                                stop=True,
                                skip_group_check=True,
                            )
                        ps_r = ps.rearrange("p (i e) -> p i e", e=64)
                        nc.scalar.copy(
                            out=v_r[0:msize, bb, mt, 8 * ch : 8 * ch + 8, 0:64],
                            in_=ps_r[0:msize, :, :],
                        )

                    def emit_proj_chunk(bb, o, outt_prev):
                        """one output-projection c-tile for batch bb (filler)"""
                        ps = ps1b_pool.tile([128, 512], F32, tag="ps1b")
                        for kk in range(CT):
                            nc.tensor.matmul(
                                ps[:, 0:N],
                                lhsT=wpt_sb[:, kk, o * 128 : (o + 1) * 128],
                                rhs=outt_prev[:, kk, :],
                                start=(kk == 0),
                                stop=(kk == CT - 1),
                                skip_group_check=True,
                            )
                        y_sb = yc_pool.tile([128, N], F32, tag="y")
                        # bias add on the ACT engine (per-partition bias)
                        nc.scalar.activation(
                            out=y_sb[:],
                            in_=ps[:, 0:N],
                            func=mybir.ActivationFunctionType.Identity,
                            bias=bp_sb[:, o : o + 1],
                            scale=1.0,
                        )
                        nc.sync.dma_start(
                            out=out_r[:, o, bb * N : (bb + 1) * N], in_=y_sb[:]
                        )

                    prev_outt = None
                    for b in range(BL):
                        bn = b * N
                        mb2 = (b % 4) * 32
                        dense_sb = dense_pool.tile([16, N], F32, tag="dense")
                        denseb_sb = dense_pool.tile([16, N], BF16, tag="denseb")
                        # x slice for the NEXT batch's V filler
                        if b < BL - 1 and V_FILLER_MODE:
                            xb_sb = xb_pool.tile([128, CT, N], BF16, tag="xb")
                            nc.sync.dma_start(
                                out=xb_sb[:],
                                in_=xt_r[:, :, (b + 1) * N : (b + 2) * N],
                            )
                        v_units = (
                            [(ch, mt) for ch in range(2) for mt in range(2)]
                            if (b < BL - 1 and V_FILLER_MODE)
                            else []
                        )
                        if V_FILLER_MODE == 1:
                            while v_units:
                                ch, mt = v_units.pop(0)
                                emit_v_unit(b + 1, ch, mt, xb_sb)
                        outt_b = outt_pool.tile([128, CT, N], BF16, tag="outt")
                        for p in range(8):
                            o = p
                            expt = expt_pool.tile([128, 3, 2, N], BF16, tag="expt")
                            for mt, (moff, msize) in enumerate(M_TILES):
                                mb = mb2 if mt == 2 else 0
                                ps_mt = pss_pool.tile([128, 2, 512], F32, tag="ps_s")
                                for hh in range(2):
                                    rb = 64 * hh
                                    nc.tensor.matmul(
                                        ps_mt[mb : mb + msize, hh, 0:N],
                                        lhsT=kt_sb[
                                            rb : rb + 64,
                                            o,
                                            bn + moff : bn + moff + msize,
                                        ],
                                        rhs=qt_sb[rb : rb + 64, o, bn : bn + N],
                                        start=True,
                                        stop=(mt != 0),
                                        tile_position=(rb, mb) if mt == 2 else None,
                                        skip_group_check=True,
                                    )
                                if mt == 0:
                                    for hh in range(2):
                                        # rank-8 prompt mask: adds -400 to
                                        # blocked (key, query) positions
                                        nc.tensor.matmul(
                                            ps_mt[0:128, hh, 0:N],
                                            lhsT=maskk_sb[0:8, 0:128],
                                            rhs=maskq_sb[0:8, 0:N],
                                            start=False,
                                            stop=True,
                                            skip_group_check=True,
                                        )
                                nc.scalar.activation(
                                    out=expt[mb : mb + msize, mt, :, :],
                                    in_=ps_mt[mb : mb + msize, 0:2, 0:N],
                                    func=mybir.ActivationFunctionType.Exp,
                                    bias=zbias_sb[0:msize, 0:1],
                                    scale=SCALE,
                                )
                            # --- PV, sums folded as 65th [v|ones] column ---
                            ps_a = pspv_pool.tile([128, 512], F32, tag="pv")
                            ps_b = pspv_pool.tile([128, 512], F32, tag="pv")
                            for hh, ps_pv in ((0, ps_a), (1, ps_b)):
                                h = 2 * p + hh
                                for mt, (moff, msize) in enumerate(M_TILES):
                                    mb = mb2 if mt == 2 else 0
                                    if mt < 2:
                                        lhsT_v = v_r[0:msize, b, mt, h, 0:65]
                                        tp = None
                                    else:
                                        lhsT_v = v2_r[mb : mb + 32, b // 4, h, 0:65]
                                        tp = (mb, 0)
                                    nc.tensor.matmul(
                                        ps_pv[0:65, 0:N],
                                        lhsT=lhsT_v,
                                        rhs=expt[mb : mb + msize, mt, hh, :],
                                        start=(mt == 0),
                                        stop=(mt == 2),
                                        skip_group_check=True,
                                        tile_position=tp,
                                    )
                            # stage to SBUF; sum rows park on partition 64
                            nc.scalar.copy(
                                out=pvstg[0:64, p, :], in_=ps_a[0:64, 0:N]
                            )
                            nc.vector.tensor_copy(
                                pvstg[64:128, p, :], ps_b[0:64, 0:N]
                            )
                            nc.vector.tensor_copy(
                                sums_stg[64:65, p, :], ps_a[64:65, 0:N]
                            )
                            nc.vector.tensor_copy(
                                sums_stg[64:65, 8 + p, :], ps_b[64:65, 0:N]
                            )
                            # PE fillers at the pair boundary
                            if p % 2 == 1 and v_units and V_FILLER_MODE >= 2:
                                ch, mt = v_units.pop(0)
                                emit_v_unit(b + 1, ch, mt, xb_sb)
                            if prev_outt is not None:
                                emit_proj_chunk(b - 1, p, prev_outt)
                        # --- batch-end: recip -> broadcast -> normalize ---
                        nc.sync.dma_start(
                            out=dense_sb[:], in_=sums_stg[64:65, :, :]
                        )
                        nc.vector.reciprocal(out=dense_sb[:], in_=dense_sb[:])
                        nc.vector.tensor_copy(denseb_sb[:], dense_sb[:])
                        # all pairs' recip rows to partitions 0/1 in one go:
                        # dense rows 0..7 = even-head recips, 8..15 = odd
                        dp = densep_pool.tile([2, 8, N], BF16, tag="dp")
                        nc.sync.dma_start(out=dp[0:1, :, :], in_=denseb_sb[0:8, :])
                        nc.sync.dma_start(out=dp[1:2, :, :], in_=denseb_sb[8:16, :])
                        for p in range(8):
                            o = p
                            psbc = ps1b_pool.tile([128, 512], F32, tag="ps1b")
                            nc.tensor.matmul(
                                psbc[:, 0:N],
                                lhsT=sel2_sb[:],
                                rhs=dp[:, p, :],
                                start=True,
                                stop=True,
                            )
                            nc.vector.tensor_tensor(
                                outt_b[:, o, :],
                                pvstg[:, p, :],
                                psbc[:, 0:N],
                                mybir.AluOpType.mult,
                            )
                        prev_outt = outt_b
                    # last batch's projection
                    for o in range(CT):
                        emit_proj_chunk(BL - 1, o, prev_outt)


    if split_waits:
        _split_multi_waits(nc)
    return nc


_NC_CACHE = {}


def _get_nc(with_vbias=True):
    if with_vbias not in _NC_CACHE:
        _NC_CACHE[with_vbias] = _build_nc(with_vbias=with_vbias)
    return _NC_CACHE[with_vbias]


def _host_inputs(x, Wqkv, bqkv, Wproj, bproj):
    bf16 = ml_dtypes.bfloat16
    shared = {}
    shared["wqkt"] = np.ascontiguousarray(Wqkv[: 2 * C].T).astype(bf16)
    shared["wvt"] = np.ascontiguousarray(Wqkv[2 * C :].T).astype(bf16)
    shared["wpt"] = np.ascontiguousarray(Wproj.T).astype(bf16)
    shared["bv"] = bqkv[2 * C :].reshape(1, C).astype(bf16)
    shared["bqk"] = np.ascontiguousarray(
        bqkv[: 2 * C].reshape(2, 8, 128).transpose(2, 0, 1).reshape(128, 16)
    ).astype(np.float32)
    shared["bp"] = np.ascontiguousarray(bproj.reshape(CT, 128).T).astype(np.float32)

    # rank-8 prompt mask: scores[k, q] += sum_j maskk[j, k] * maskq[j, q]
    # blocked (key k < 32, query q): q >= 32 or q < 4*(k//4) -> -400
    # (exp(-400/8) = exp(-50) ~ 0)
    j_ = np.arange(8)[:, None]
    k_ = np.arange(N)[None, :]
    q_ = np.arange(N)[None, :]
    maskk = ((k_ // 4 == j_) & (k_ < 32)).astype(np.float32)
    maskq = np.where((q_ >= 32) | (q_ < 4 * j_), -400.0, 0.0).astype(np.float32)
    shared["maskk"] = maskk.astype(bf16)
    shared["maskq"] = maskq.astype(bf16)

    # broadcast selector: dp row 0 (even-head recip) -> rows 0..63,
    # dp row 1 (odd-head recip) -> rows 64..127
    sel2 = np.zeros((2, 128), bf16)
    sel2[0, 0:64] = 1.0
    sel2[1, 64:128] = 1.0
    shared["sel2"] = sel2

    in_maps = []
    for i in range(8):
        xc = x[:, i * BL : (i + 1) * BL, :]  # (N, BL, C)
        xt = np.ascontiguousarray(xc.transpose(2, 1, 0).reshape(C, T)).astype(bf16)
        m = dict(shared)
        m["xt"] = xt
        in_maps.append(m)
    return in_maps


def kernel(x, Wqkv, bqkv, Wproj, bproj):
    x = np.asarray(x, dtype=np.float32)
    Wqkv = np.asarray(Wqkv, dtype=np.float32)
    bqkv = np.asarray(bqkv, dtype=np.float32)
    Wproj = np.asarray(Wproj, dtype=np.float32)
    bproj = np.asarray(bproj, dtype=np.float32)

    # the staged inputs carry structurally-zero biases; skip the V-bias
    # rank-1 matmuls (~15us of PE) unless a nonzero bias is present
    nc = _get_nc(with_vbias=bool(np.any(bqkv[2 * C :])))
    in_maps = _host_inputs(x, Wqkv, bqkv, Wproj, bproj)
    res = run_bass_kernel_spmd(nc, in_maps, core_ids=list(range(8)))

    full = np.empty((N, 64, C), dtype=np.float32)
    for i in range(8):
        yT = np.asarray(res.results[i]["out"], dtype=np.float32)  # [C, T]
        full[:, i * BL : (i + 1) * BL, :] = yT.reshape(C, BL, N).transpose(2, 1, 0)
    return full
